# revision 11
# baseline (speedup 1.0000x reference)
# Llama attention layer (B=2, S=2048, H=4096, 32 q-heads / 8 kv-heads, HD=128,
# RoPE, causal SDPA, o_proj) on 8 Trainium2 NeuronCores.
#
# Sharding: 2-way data parallel over batch x 4-way tensor parallel over heads.
#   core c: batch b = c // 4, TP rank t = c % 4
#   rank t owns q-heads [8t, 8t+8), kv-heads [2t, 2t+2),
#   Wq/Wk/Wv column slices, Wo row slice. Each core emits a partial o_proj
#   output [S, H]; the host sums the 4 TP partials per batch (the all-reduce).
#
# Device kernel layout choices (all fp32 — TensorE streams 1 elem/cell/cycle
# regardless of dtype, so fp32 matmul runs at full rate):
#   - host supplies hidden^T [H, S] per batch (fp32 has no DMA-transpose)
#   - Q/K produced transposed [hd, s]; RoPE's rotate_half is one PE matmul
#     with a +-1 permutation matrix; cos/sin tables host-precomputed [hd, S]
#   - V produced natural [s, hd] (lhsT = hidden^T tiles)
#   - scores computed transposed [sk, sq] = K @ Q^T; max-free softmax
#     (scores are O(10) here, exp is safe in fp32); causal handled by
#     skipping sk>sq tiles + one [128,128] additive mask on the diagonal
#   - rowsum via ones-vector matmul accumulated in PSUM; normalization via
#     DVE reciprocal + GpSimd partition_broadcast
#   - attn out^T staged to a DRAM scratch, then o_proj in column-quarters
#     (keeps the Wo working set small)

import numpy as np
from contextlib import ExitStack

# ---- problem constants (hardcoded per contract) ----
B, S, H = 2, 2048, 4096
NH, NKV, HD = 32, 8, 128
ROPE_THETA = 10000.0
N_CORES = 8
DP, TP = 2, 4
QH_C = NH // TP        # 8 q-heads per core
KVH_C = NKV // TP      # 2 kv-heads per core
GROUPS = NH // NKV     # 4

F32 = None  # set lazily (mybir.dt.float32)


def _emit(tc, t, cfg):
    """Emit the per-core program. `t` is a dict of DRAM APs.

    cfg: dict with S, H, QH, KVH, S_BLK (sq block width, multiple of 128,
    <=512), N_QTR (o_proj column-split count).
    """
    import concourse.bass as bass
    from concourse import mybir

    nc = tc.nc
    f32 = mybir.dt.float32
    S_, H_, QH, KVH = cfg["S"], cfg["H"], cfg["QH"], cfg["KVH"]
    S_BLK = cfg["S_BLK"]
    N_QTR = cfg["N_QTR"]
    GRP = QH // KVH
    KT_ = H_ // 128            # contraction tiles for projections
    NBLK = S_ // S_BLK         # sq blocks
    BSUB = S_BLK // 128        # 128-wide subcolumns per block
    NSK_TOT = S_ // 128        # total sk tiles
    DQ = QH * 128              # q out dim per core
    DKV = KVH * 128
    HOUT = H_                  # o_proj out dim
    QTR = HOUT // N_QTR        # o_proj column chunk held resident
    assert QTR % 512 == 0 or QTR <= 512
    N_OP = min(512, QTR)       # matmul N for o_proj
    SCALE = 1.0 / float(np.sqrt(HD))

    ctx = tc._emit_ctx  # ExitStack owned by caller

    # ---------------- pools ----------------
    singles = ctx.enter_context(tc.tile_pool(name="singles", bufs=1))
    hT_pool = ctx.enter_context(tc.tile_pool(name="hT", bufs=KT_ + 1))
    w_pool = ctx.enter_context(tc.tile_pool(name="w", bufs=8))
    trig = ctx.enter_context(tc.tile_pool(name="trig", bufs=2))
    qstage = ctx.enter_context(tc.tile_pool(name="qstage", bufs=2))
    rope_tmp = ctx.enter_context(tc.tile_pool(name="rope", bufs=2))
    expp = ctx.enter_context(tc.tile_pool(name="expp", bufs=3))
    attn_mis = ctx.enter_context(tc.tile_pool(name="attnm", bufs=2))
    psum = ctx.enter_context(tc.tile_pool(name="psum", bufs=8, space="PSUM"))
    dram = ctx.enter_context(tc.tile_pool(name="dram", bufs=1, space="DRAM"))
    c_at = ctx.enter_context(tc.tile_pool(name="c_at", bufs=10))
    c_wo = ctx.enter_context(tc.tile_pool(name="c_wo", bufs=QH))
    c_out = ctx.enter_context(tc.tile_pool(name="c_out", bufs=2))

    # ---------------- persistent tiles ----------------
    from concourse.masks import make_identity

    KT_all = singles.tile([128, KVH, S_], f32)          # K^T roped
    V_all = singles.tile([128, NSK_TOT, DKV], f32)      # V natural per sk tile
    RT_sb = singles.tile([128, 128], f32)
    mask_sb = singles.tile([128, 128], f32)
    ident_sb = singles.tile([128, 128], f32)
    ones_sb = singles.tile([128, 1], f32)
    nc.vector.memset(ones_sb, 1.0)
    make_identity(nc, ident_sb)
    nc.sync.dma_start(RT_sb, t["RT"])
    nc.sync.dma_start(mask_sb, t["maskT"])

    attnT = dram.tile([QH, 128, S_], f32)               # attn out^T scratch

    # ---------------- phase A+B: per sq block ----------------
    for blk in range(NBLK):
        s0 = blk * S_BLK
        # hidden^T tiles for this block
        hT = []
        for k in range(KT_):
            ht = hT_pool.tile([128, S_BLK], f32, tag="hT")
            nc.sync.dma_start(ht, t["hiddenT"][k * 128:(k + 1) * 128, s0:s0 + S_BLK])
            hT.append(ht)
        cos_b = trig.tile([128, S_BLK], f32, tag="cos")
        sin_b = trig.tile([128, S_BLK], f32, tag="sin")
        nc.sync.dma_start(cos_b, t["cosT"][:, s0:s0 + S_BLK])
        nc.sync.dma_start(sin_b, t["sinT"][:, s0:s0 + S_BLK])

        def project_T(w_ap, m, dst, rope):
            """Project one 128-wide out-dim tile, transposed: dst[hd, S_BLK].
            lhsT = W[:, m*128:+128] k-tiles, rhs = hidden^T tiles."""
            ps = psum.tile([128, S_BLK], f32, tag="ps")
            for k in range(KT_):
                wt = w_pool.tile([128, 128], f32, tag="wt")
                nc.sync.dma_start(wt, w_ap[k * 128:(k + 1) * 128, m * 128:(m + 1) * 128])
                nc.tensor.matmul(ps, wt, hT[k], start=(k == 0), stop=(k == KT_ - 1))
            if not rope:
                nc.vector.tensor_copy(dst, ps)
                return
            raw = qstage.tile([128, S_BLK], f32, tag="qraw")
            nc.vector.tensor_copy(raw, ps)
            rot = psum.tile([128, S_BLK], f32, tag="ps")
            nc.tensor.matmul(rot, RT_sb, raw, start=True, stop=True)
            t1 = rope_tmp.tile([128, S_BLK], f32, tag="t1")
            nc.vector.tensor_mul(t1, raw, cos_b)
            t2 = rope_tmp.tile([128, S_BLK], f32, tag="t2")
            nc.vector.tensor_mul(t2, rot, sin_b)
            nc.vector.tensor_add(dst, t1, t2)

        # K^T (roped) for this block's tokens
        for m in range(KVH):
            project_T(t["Wk"], m, KT_all[:, m, s0:s0 + S_BLK], rope=True)
        # V^T, then PE-transpose 128x128 blocks into natural layout
        for m in range(KVH):
            vt = rope_tmp.tile([128, S_BLK], f32, tag="vt")
            project_T(t["Wv"], m, vt, rope=False)
            for c in range(BSUB):
                tps = psum.tile([128, 128], f32, tag="ps")
                nc.tensor.transpose(tps, vt[:, c * 128:(c + 1) * 128], ident_sb)
                nc.vector.tensor_copy(
                    V_all[:, blk * BSUB + c, m * 128:(m + 1) * 128], tps)

        # q heads: project + attention
        nsk = (blk + 1) * BSUB  # sk tiles needed (causal)
        for h in range(QH):
            kv = h // GRP
            qT = qstage.tile([128, S_BLK], f32, tag="qT")
            project_T(t["Wq"], h, qT, rope=True)

            pv = psum.tile([128, S_BLK], f32, tag="ps")     # out^T unnorm
            rs = psum.tile([1, S_BLK], f32, tag="ps")       # rowsum
            for i in range(nsk):
                r = i - blk * BSUB  # >=0 -> diagonal band
                off = max(r, 0) * 128
                sps = psum.tile([128, S_BLK], f32, tag="ps")
                nc.tensor.matmul(
                    sps[:, off:], KT_all[:, kv, i * 128:(i + 1) * 128],
                    qT[:, off:], start=True, stop=True)
                if r >= 0:
                    nc.vector.tensor_add(
                        sps[:, off:off + 128], sps[:, off:off + 128], mask_sb)
                e = expp.tile([128, S_BLK], f32, tag="e")
                if off > 0:
                    nc.gpsimd.memset(e[:, :off], 0.0)
                nc.scalar.activation(
                    e[:, off:], sps[:, off:],
                    func=mybir.ActivationFunctionType.Exp, scale=SCALE)
                nc.tensor.matmul(rs, ones_sb, e, start=(i == 0), stop=(i == nsk - 1))
                nc.tensor.matmul(
                    pv, V_all[:, i, kv * 128:(kv + 1) * 128], e,
                    start=(i == 0), stop=(i == nsk - 1))
            recip = attn_mis.tile([1, S_BLK], f32, tag="recip")
            nc.vector.reciprocal(recip, rs)
            rb = attn_mis.tile([128, S_BLK], f32, tag="rb")
            nc.gpsimd.partition_broadcast(rb, recip)
            o = attn_mis.tile([128, S_BLK], f32, tag="o")
            nc.vector.tensor_mul(o, pv, rb)
            nc.sync.dma_start(attnT[h, :, s0:s0 + S_BLK], o)

    # ---------------- phase C: o_proj (partial over this core's heads) ------
    NSQ = S_ // 128
    for qtr in range(N_QTR):
        q0 = qtr * QTR
        wo = []
        for h in range(QH):
            wt = c_wo.tile([128, QTR], f32, tag="wo")
            nc.sync.dma_start(wt, t["Wo"][h * 128:(h + 1) * 128, q0:q0 + QTR])
            wo.append(wt)
        for sq in range(NSQ):
            ats = []
            for h in range(QH):
                at = c_at.tile([128, 128], f32, tag="at")
                nc.sync.dma_start(at, attnT[h, :, sq * 128:(sq + 1) * 128])
                ats.append(at)
            outt = c_out.tile([128, QTR], f32, tag="co")
            for n in range(QTR // N_OP):
                ps = psum.tile([128, N_OP], f32, tag="ps")
                for h in range(QH):
                    nc.tensor.matmul(
                        ps, ats[h], wo[h][:, n * N_OP:(n + 1) * N_OP],
                        start=(h == 0), stop=(h == QH - 1))
                nc.scalar.copy(outt[:, n * N_OP:(n + 1) * N_OP], ps)
            nc.sync.dma_start(
                t["outp"][sq * 128:(sq + 1) * 128, q0:q0 + QTR], outt)


def _build(cfg):
    """Build + compile the Bass program for one core (SPMD across 8)."""
    import concourse.bass as bass
    import concourse.tile as tile
    from concourse import bacc, mybir

    f32 = mybir.dt.float32
    S_, H_, QH, KVH = cfg["S"], cfg["H"], cfg["QH"], cfg["KVH"]

    nc = bacc.Bacc("TRN2", target_bir_lowering=False, debug=False,
                   num_devices=cfg["n_cores"])
    t = {}
    t["hiddenT"] = nc.dram_tensor("hiddenT", [H_, S_], f32, kind="ExternalInput").ap()
    t["Wq"] = nc.dram_tensor("Wq", [H_, QH * 128], f32, kind="ExternalInput").ap()
    t["Wk"] = nc.dram_tensor("Wk", [H_, KVH * 128], f32, kind="ExternalInput").ap()
    t["Wv"] = nc.dram_tensor("Wv", [H_, KVH * 128], f32, kind="ExternalInput").ap()
    t["Wo"] = nc.dram_tensor("Wo", [QH * 128, H_], f32, kind="ExternalInput").ap()
    t["cosT"] = nc.dram_tensor("cosT", [128, S_], f32, kind="ExternalInput").ap()
    t["sinT"] = nc.dram_tensor("sinT", [128, S_], f32, kind="ExternalInput").ap()
    t["RT"] = nc.dram_tensor("RT", [128, 128], f32, kind="ExternalInput").ap()
    t["maskT"] = nc.dram_tensor("maskT", [128, 128], f32, kind="ExternalInput").ap()
    t["outp"] = nc.dram_tensor("outp", [S_, H_], f32, kind="ExternalOutput").ap()

    with tile.TileContext(nc) as tc:
        with ExitStack() as ectx:
            tc._emit_ctx = ectx
            _emit(tc, t, cfg)
    nc.compile()
    return nc


def _host_tables(position_ids_b):
    """cos/sin tables [HD, S] + rotation matrix + diagonal causal mask."""
    pos = np.asarray(position_ids_b, dtype=np.float64)  # [S]
    inv = 1.0 / (ROPE_THETA ** (np.arange(0, HD, 2, dtype=np.float64) / HD))  # [64]
    fr = pos[:, None] * inv[None, :]                    # [S, 64]
    emb = np.concatenate([fr, fr], axis=1)              # [S, HD]
    cosT = np.ascontiguousarray(np.cos(emb).T.astype(np.float32))  # [HD, S]
    sinT = np.ascontiguousarray(np.sin(emb).T.astype(np.float32))
    return cosT, sinT


def _rot_matrix():
    RT = np.zeros((HD, HD), dtype=np.float32)
    half = HD // 2
    idx = np.arange(half)
    RT[idx, idx + half] = 1.0   # rot[d>=64] = q[d-64]
    RT[idx + half, idx] = -1.0  # rot[d<64] = -q[d+64]
    return RT


def _diag_mask():
    # scores^T diag sub-tile [sk_local(p), sq_local(f)]: mask where p > f
    m = np.zeros((128, 128), dtype=np.float32)
    p = np.arange(128)[:, None]
    f = np.arange(128)[None, :]
    m[p > f] = -1.0e30
    return m


_CACHE = {}

FULL_CFG = dict(S=S, H=H, QH=QH_C, KVH=KVH_C, S_BLK=512, N_QTR=8, n_cores=N_CORES)


def _get_nc():
    if "full" not in _CACHE:
        _CACHE["full"] = _build(FULL_CFG)
    return _CACHE["full"]


def kernel(hidden_states, position_ids, Wq, Wk, Wv, Wo):
    out, _ = _run(hidden_states, position_ids, Wq, Wk, Wv, Wo)
    return out


def _run(hidden_states, position_ids, Wq, Wk, Wv, Wo, **spmd_kwargs):
    from concourse.bass_utils import run_bass_kernel_spmd

    hidden_states = np.asarray(hidden_states, dtype=np.float32)
    Wq = np.asarray(Wq, dtype=np.float32)
    Wk = np.asarray(Wk, dtype=np.float32)
    Wv = np.asarray(Wv, dtype=np.float32)
    Wo = np.asarray(Wo, dtype=np.float32)

    nc = _get_nc()
    RT = _rot_matrix()
    maskT = _diag_mask()

    in_maps = []
    per_b = {}
    for b in range(DP):
        hT = np.ascontiguousarray(hidden_states[b].T)          # [H, S]
        cosT, sinT = _host_tables(position_ids[b])
        per_b[b] = (hT, cosT, sinT)
    for c in range(N_CORES):
        b, t = c // TP, c % TP
        hT, cosT, sinT = per_b[b]
        in_maps.append({
            "hiddenT": hT,
            "Wq": np.ascontiguousarray(Wq[:, t * QH_C * HD:(t + 1) * QH_C * HD]),
            "Wk": np.ascontiguousarray(Wk[:, t * KVH_C * HD:(t + 1) * KVH_C * HD]),
            "Wv": np.ascontiguousarray(Wv[:, t * KVH_C * HD:(t + 1) * KVH_C * HD]),
            "Wo": np.ascontiguousarray(Wo[t * QH_C * HD:(t + 1) * QH_C * HD, :]),
            "cosT": cosT, "sinT": sinT, "RT": RT, "maskT": maskT,
        })

    res = run_bass_kernel_spmd(nc, in_maps, core_ids=list(range(N_CORES)),
                               **spmd_kwargs)
    out = np.zeros((B, S, H), dtype=np.float32)
    for c in range(N_CORES):
        out[c // TP] += res.results[c]["outp"]
    return out, res
